# revision 1
# baseline (speedup 1.0000x reference)
"""Causal self-attention on 8 trn2 NeuronCores.

Sharding: core c = (batch b = c//2, head-group g = c%2). Each core computes
QKV projection for its 8 heads of its batch, causal flash-attention in a
transposed (S^T) layout, and a partial out-projection (its 512 rows of
w_out). Host sums the two partials per batch and adds b_out.

All matmuls run as float32r (full-rate fp32 PE mode, moving dim 512; all
operand tiles are declared float32r so producers round on write, which the
walrus verifier requires). Softmax skips max-subtraction (logits ~N(0,1))
and each head's V tile carries a ones column so the PV matmul also yields
the softmax denominator. The kernel is a single interleaved pass —
projection chunk i, attention q-chunk i, out-projection i — so PSUM slot
reuse follows PE program order and the scheduler can overlap phases.
Attention processes heads in pairs at partition bases 0/64: the K=64 S^T
matmuls of a pair land in distinct PE row groups and run concurrently.
"""

import numpy as np

B = 4
T = 2048
C = 1024
HG = 512          # head channels per core (8 heads x 64)
Dh = 64
NHL = 8           # local heads per core
TCH = 512         # T-chunk (q-chunk) width
NTC = T // TCH    # 4
NCC = C // 128    # 8 contraction chunks for projections
NMT = HG // 128   # 4 row-tiles of q/k channels
VW = NHL * (Dh + 1)   # 520: V tiles with a ones column per head

_CACHE = {}


def _build_nc():
    import concourse.bass as bass
    import concourse.bacc as bacc
    import concourse.tile as tile
    import concourse.mybir as mybir

    f32 = mybir.dt.float32
    f32r = mybir.dt.float32r
    bf16 = mybir.dt.bfloat16
    AF = mybir.ActivationFunctionType

    nc = bacc.Bacc("TRN2", target_bir_lowering=False, debug=False,
                   enable_asserts=False)
    x_d = nc.dram_tensor("x", [T, C], f32r, kind="ExternalInput").ap()
    wq_d = nc.dram_tensor("wq", [C, HG], f32r, kind="ExternalInput").ap()
    wk_d = nc.dram_tensor("wk", [C, HG], f32r, kind="ExternalInput").ap()
    wv_d = nc.dram_tensor("wv", [C, HG], f32r, kind="ExternalInput").ap()
    bq_d = nc.dram_tensor("bq", [HG], f32, kind="ExternalInput").ap()
    bk_d = nc.dram_tensor("bk", [HG], f32, kind="ExternalInput").ap()
    bv_d = nc.dram_tensor("bv", [HG], f32, kind="ExternalInput").ap()
    wo_d = nc.dram_tensor("wo", [HG, C], f32r, kind="ExternalInput").ap()
    mk_d = nc.dram_tensor("mask", [2, 128, 2 * TCH], bf16,
                          kind="ExternalInput").ap()
    id_d = nc.dram_tensor("ident", [128, 128], f32r, kind="ExternalInput").ap()
    on_d = nc.dram_tensor("ones", [1, 64], f32r, kind="ExternalInput").ap()
    vo_d = nc.dram_tensor("vones", [128, NHL], f32r, kind="ExternalInput").ap()
    y_d = nc.dram_tensor("y", [T, C], f32, kind="ExternalOutput").ap()

    def mm(out, lhsT, rhs, start, stop):
        nc.tensor.matmul(out, lhsT, rhs, start=start, stop=stop)

    with tile.TileContext(nc) as tc:
        with tc.tile_pool(name="wp", bufs=1) as wp, \
             tc.tile_pool(name="ktp", bufs=1) as ktp, \
             tc.tile_pool(name="vp", bufs=1) as vp, \
             tc.tile_pool(name="qtp", bufs=1) as qtp, \
             tc.tile_pool(name="xp", bufs=1) as xp, \
             tc.tile_pool(name="xtp", bufs=1) as xtp, \
             tc.tile_pool(name="esp", bufs=3) as esp, \
             tc.tile_pool(name="rp", bufs=1) as rp, \
             tc.tile_pool(name="otp", bufs=1) as otp, \
             tc.tile_pool(name="yst", bufs=2) as yst, \
             tc.tile_pool(name="psM", bufs=2, space="PSUM") as psM, \
             tc.tile_pool(name="psS", bufs=2, space="PSUM") as psS, \
             tc.tile_pool(name="psO", bufs=2, space="PSUM") as psO:

            # ---- constants (sync/HWDGE: needed first) ----
            ident = wp.tile([128, 128], f32r, tag="ident")
            nc.sync.dma_start(out=ident, in_=id_d)
            ones = wp.tile([1, 64], f32r, tag="ones")
            nc.sync.dma_start(out=ones, in_=on_d)
            # prefetch chunk-0 x rows ahead of every weight DMA
            xin0 = [xp.tile([128, C], f32r, name=f"xin{s}", tag=f"xin{s}")
                    for s in range(4)]
            for s in range(4):
                nc.sync.dma_start(out=xin0[s], in_=x_d[s * 128:(s + 1) * 128, :])

            # ---- weights etc on the gpsimd/SWDGE queues so the x-chunk
            # loads (sync/HWDGE) aren't queued behind 26MB of weights ----
            wq_sb = [wp.tile([128, HG], f32r, name=f"wq{c}", tag=f"wq{c}")
                     for c in range(NCC)]
            wk_sb = [wp.tile([128, HG], f32r, name=f"wk{c}", tag=f"wk{c}")
                     for c in range(NCC)]
            wv_sb = [wp.tile([128, HG], f32r, name=f"wv{c}", tag=f"wv{c}")
                     for c in range(NCC)]
            for c in range(NCC):
                nc.sync.dma_start(out=wq_sb[c], in_=wq_d[c * 128:(c + 1) * 128, :])
            for c in range(NCC):
                nc.gpsimd.dma_start(out=wk_sb[c], in_=wk_d[c * 128:(c + 1) * 128, :])
            for c in range(NCC):
                nc.gpsimd.dma_start(out=wv_sb[c], in_=wv_d[c * 128:(c + 1) * 128, :])
            wo_sb = [wp.tile([128, C], f32r, name=f"wo{m}", tag=f"wo{m}")
                     for m in range(NMT)]
            for m in range(NMT):
                nc.gpsimd.dma_start(out=wo_sb[m], in_=wo_d[m * 128:(m + 1) * 128, :])
            bq_sb = [wp.tile([128, 1], f32, name=f"bq{m}", tag=f"bq{m}")
                     for m in range(NMT)]
            bk_sb = [wp.tile([128, 1], f32, name=f"bk{m}", tag=f"bk{m}")
                     for m in range(NMT)]
            for m in range(NMT):
                nc.gpsimd.dma_start(
                    out=bq_sb[m],
                    in_=bq_d[m * 128:(m + 1) * 128].rearrange("(p o) -> p o", o=1))
                nc.gpsimd.dma_start(
                    out=bk_sb[m],
                    in_=bk_d[m * 128:(m + 1) * 128].rearrange("(p o) -> p o", o=1))
            bv_bc = wp.tile([128, HG], f32, tag="bvbc")
            bv_src = bass.AP(tensor=bv_d.tensor, offset=bv_d.offset,
                             ap=[[0, 128]] + list(bv_d.ap))
            nc.gpsimd.dma_start(out=bv_bc, in_=bv_src)
            masks = [wp.tile([128, 2 * TCH], bf16, name=f"mk{j}", tag=f"mk{j}")
                     for j in range(2)]
            for j in range(2):
                nc.gpsimd.dma_start(out=masks[j], in_=mk_d[j])

            kt_sb = [ktp.tile([128, T], f32r, name=f"kt{m}", tag=f"kt{m}")
                     for m in range(NMT)]
            v_sb = [vp.tile([128, VW], f32r, name=f"v{t}", tag=f"v{t}")
                    for t in range(T // 128)]
            for t in range(T // 128):
                nc.gpsimd.dma_start(
                    out=v_sb[t].rearrange("p (h e) -> p h e", h=NHL)[:, :, Dh:Dh + 1],
                    in_=vo_d.rearrange("p (h e) -> p h e", e=1))

            # ======== software-pipelined pass: projection work for chunk
            # ti+1 is emitted as PE "filler" between attention head-pairs of
            # q-chunk ti, so the PE never idles a full HAM window and the
            # clock stays at 2.4 GHz ========
            store = {}

            def proj_units(ti):
                t0 = ti * TCH
                st = store[ti] = {}

                def u_load():
                    if ti == 0:
                        st["xin"] = xin0
                    else:
                        xin = st["xin"] = [
                            xp.tile([128, C], f32r, name=f"xin{s}", tag=f"xin{s}")
                            for s in range(4)]
                        for s in range(4):
                            nc.sync.dma_start(
                                out=xin[s],
                                in_=x_d[t0 + s * 128:t0 + (s + 1) * 128, :])
                    st["xt"] = [None] * NCC
                    st["qt"] = [None] * NMT
                yield u_load

                def u_tr(c):
                    def f():
                        ptr = psM.tile([128, TCH], f32, tag="mm", name="ptr")
                        for s in range(4):
                            nc.tensor.transpose(
                                ptr[:, s * 128:(s + 1) * 128].bitcast(f32r),
                                st["xin"][s][:, c * 128:(c + 1) * 128], ident)
                        xt = xtp.tile([128, TCH], f32r, name=f"xt{c}",
                                      tag=f"xt{c}")
                        st["xt"][c] = xt
                        nc.vector.tensor_copy(xt, ptr)
                    return f
                for c in range(NCC):
                    yield u_tr(c)

                def u_pq(m):
                    def f():
                        pq = psM.tile([128, TCH], f32, tag="mm", name="pq")
                        for c in range(NCC):
                            mm(pq, wq_sb[c][:, m * 128:(m + 1) * 128],
                               st["xt"][c], c == 0, c == NCC - 1)
                        qtm = qtp.tile([128, TCH], f32r, name=f"qt{m}",
                                       tag=f"qt{m}")
                        st["qt"][m] = qtm
                        nc.vector.tensor_scalar_add(qtm, pq, bq_sb[m])
                    return f
                for m in range(NMT):
                    yield u_pq(m)

                def u_pk(m):
                    def f():
                        pk = psM.tile([128, TCH], f32, tag="mm", name="pk")
                        for c in range(NCC):
                            mm(pk, wk_sb[c][:, m * 128:(m + 1) * 128],
                               st["xt"][c], c == 0, c == NCC - 1)
                        nc.vector.tensor_scalar_add(
                            kt_sb[m][:, t0:t0 + TCH], pk, bk_sb[m])
                    return f
                for m in range(NMT):
                    yield u_pk(m)

                def u_pv(s):
                    def f():
                        pv = psM.tile([128, HG], f32, tag="mm", name="pv")
                        for c in range(NCC):
                            mm(pv, st["xt"][c][:, s * 128:(s + 1) * 128],
                               wv_sb[c], c == 0, c == NCC - 1)
                        vt = v_sb[(t0 + s * 128) // 128]
                        nc.vector.tensor_add(
                            vt.rearrange("p (h e) -> p h e", h=NHL)[:, :, 0:Dh],
                            pv.rearrange("p (h d) -> p h d", h=NHL),
                            bv_bc.rearrange("p (h d) -> p h d", h=NHL))
                    return f
                for s in range(4):
                    yield u_pv(s)

            from collections import deque
            units = deque(proj_units(0))
            for ti in range(NTC):
                while units:          # finish chunk ti's projections
                    units.popleft()()
                units = deque(proj_units(ti + 1)) if ti + 1 < NTC else deque()

                qc = ti
                nkt = 4 * (qc + 1)
                ngrp = nkt // 2
                qt = store[ti]["qt"]
                fill_per_pair = (len(units) + NMT - 1) // NMT if units else 0
                ot = [otp.tile([128, TCH], f32r, name=f"ot{m}", tag=f"ot{m}")
                      for m in range(NMT)]
                for hp in range(NMT):
                    h0, h1 = 2 * hp, 2 * hp + 1
                    qt0 = qt[hp][0:64, :]
                    qt1 = qt[hp][64:128, :]
                    po0 = psO.tile([128, TCH], f32, tag="o", name="po0")
                    po1 = psO.tile([128, TCH], f32, tag="o", name="po1")
                    for grp in range(ngrp):
                        k0, k1 = 2 * grp, 2 * grp + 1
                        ps0 = psS.tile([128, 2 * TCH], f32, tag="sT", name="ps0")
                        ps1 = psS.tile([128, 2 * TCH], f32, tag="sT", name="ps1")
                        # adjacent pair matmuls -> distinct PE row groups
                        mm(ps0[:, 0:TCH],
                           kt_sb[hp][0:64, k0 * 128:(k0 + 1) * 128], qt0,
                           True, True)
                        mm(ps1[:, 0:TCH],
                           kt_sb[hp][64:128, k0 * 128:(k0 + 1) * 128], qt1,
                           True, True)
                        mm(ps0[:, TCH:],
                           kt_sb[hp][0:64, k1 * 128:(k1 + 1) * 128], qt0,
                           True, True)
                        mm(ps1[:, TCH:],
                           kt_sb[hp][64:128, k1 * 128:(k1 + 1) * 128], qt1,
                           True, True)
                        es0 = esp.tile([128, 2 * TCH], f32r, tag="es", name="es0")
                        es1 = esp.tile([128, 2 * TCH], f32r, tag="es", name="es1")
                        nc.scalar.activation(es0, ps0, AF.Exp, scale=0.125)
                        nc.scalar.activation(es1, ps1, AF.Exp, scale=0.125)
                        dj = grp - (ngrp - 2)   # 0/1 for the diagonal groups
                        if dj >= 0:
                            nc.vector.tensor_mul(es0, es0, masks[dj])
                            nc.vector.tensor_mul(es1, es1, masks[dj])
                        mm(po0[0:Dh + 1, :], v_sb[k0][:, h0 * 65:(h0 + 1) * 65],
                           es0[:, 0:TCH], k0 == 0, False)
                        mm(po1[0:Dh + 1, :], v_sb[k0][:, h1 * 65:(h1 + 1) * 65],
                           es1[:, 0:TCH], k0 == 0, False)
                        mm(po0[0:Dh + 1, :], v_sb[k1][:, h0 * 65:(h0 + 1) * 65],
                           es0[:, TCH:], False, k1 == nkt - 1)
                        mm(po1[0:Dh + 1, :], v_sb[k1][:, h1 * 65:(h1 + 1) * 65],
                           es1[:, TCH:], False, k1 == nkt - 1)
                    # normalize: broadcast D via PE, fast-reciprocal, scale
                    for h, po in ((h0, po0), (h1, po1)):
                        base = (h % 2) * 64
                        dsb = rp.tile([1, TCH], f32r, tag="rc", name="dsb")
                        nc.vector.tensor_copy(dsb, po[Dh:Dh + 1, :])
                        rb = psM.tile([128, TCH], f32, tag="mm", name="rb")
                        mm(rb[0:Dh, :], ones, dsb, True, True)
                        rbs = rp.tile([Dh, TCH], f32, tag="rbs", name="rbs")
                        nc.vector.reciprocal_approx_fast(rbs, rb[0:Dh, :])
                        nc.vector.tensor_mul(
                            ot[hp][base:base + 64, :], po[0:Dh, :], rbs)
                    # PE filler: next chunk's projection work keeps HAM warm
                    for _ in range(min(fill_per_pair, len(units))):
                        units.popleft()()

                # ======== out-projection for this q-chunk ========
                for s in range(4):
                    for n in range(2):
                        py = psM.tile([128, TCH], f32, tag="mm", name="py")
                        for m in range(NMT):
                            mm(py, ot[m][:, s * 128:(s + 1) * 128],
                               wo_sb[m][:, n * TCH:(n + 1) * TCH],
                               m == 0, m == NMT - 1)
                        yt = yst.tile([128, TCH], f32, tag="yst", name="yt")
                        nc.vector.tensor_copy(yt, py)
                        nc.sync.dma_start(
                            out=y_d[qc * TCH + s * 128:qc * TCH + (s + 1) * 128,
                                    n * TCH:(n + 1) * TCH],
                            in_=yt)
    nc.compile()
    return nc


def _get_nc():
    if "nc" not in _CACHE:
        _CACHE["nc"] = _build_nc()
    return _CACHE["nc"]


def _masks_np():
    import ml_dtypes
    p = np.arange(128)[:, None]
    f = np.arange(TCH)[None, :]
    quads = [(f >= j * 128 + p).astype(np.float32) for j in range(4)]
    pairs = [np.concatenate([quads[0], quads[1]], axis=1),
             np.concatenate([quads[2], quads[3]], axis=1)]
    return np.stack(pairs).astype(ml_dtypes.bfloat16)


def _in_maps(x, w_qkv, b_qkv, w_out):
    masks = _masks_np()
    maps = []
    for c in range(8):
        b, g = c // 2, c % 2
        s = g * HG
        maps.append({
            "x": np.ascontiguousarray(x[b]),
            "wq": np.ascontiguousarray(w_qkv[:, s:s + HG]),
            "wk": np.ascontiguousarray(w_qkv[:, C + s:C + s + HG]),
            "wv": np.ascontiguousarray(w_qkv[:, 2 * C + s:2 * C + s + HG]),
            "bq": np.ascontiguousarray(b_qkv[s:s + HG]),
            "bk": np.ascontiguousarray(b_qkv[C + s:C + s + HG]),
            "bv": np.ascontiguousarray(b_qkv[2 * C + s:2 * C + s + HG]),
            "wo": np.ascontiguousarray(w_out[s:s + HG, :]),
            "mask": masks,
            "ident": np.eye(128, dtype=np.float32),
            "ones": np.ones((1, 64), dtype=np.float32),
            "vones": np.ones((128, NHL), dtype=np.float32),
        })
    return maps


def _run(x, w_qkv, b_qkv, w_out, b_out, trace=False, tmpdir=None):
    from concourse import bass_utils
    nc = _get_nc()
    maps = _in_maps(x, w_qkv, b_qkv, w_out)
    # the device occasionally reports a transient unrecoverable-exec error
    # right after a reset; one retry clears it
    last = None
    for attempt in range(3):
        try:
            res = bass_utils.run_bass_kernel_spmd(
                nc, maps, core_ids=list(range(8)), trace=trace, tmpdir=tmpdir)
            break
        except Exception as e:
            last = e
            if attempt == 2:
                raise
    else:
        raise last
    ys = [res.results[c]["y"] for c in range(8)]
    out = np.stack([ys[2 * b] + ys[2 * b + 1] for b in range(B)])
    out += np.asarray(b_out, dtype=np.float32)[None, None, :]
    return out.astype(np.float32), res


def kernel(x, w_qkv, b_qkv, w_out, b_out):
    x = np.asarray(x, dtype=np.float32)
    w_qkv = np.asarray(w_qkv, dtype=np.float32)
    b_qkv = np.asarray(b_qkv, dtype=np.float32)
    w_out = np.asarray(w_out, dtype=np.float32)
    b_out = np.asarray(b_out, dtype=np.float32)
    out, _ = _run(x, w_qkv, b_qkv, w_out, b_out, trace=False)
    return out



# revision 17
# speedup vs baseline: 1.3507x; 1.3507x over previous
"""Causal self-attention on 8 trn2 NeuronCores.

Sharding: core c = (batch b = c//2, head-group g = c%2). Each core computes
QKV projection for its 8 heads of its batch, causal flash-attention in a
transposed (S^T) layout, and a partial out-projection (its 512 rows of
w_out). Host sums the two partials per batch and adds b_out.

v4 design:
- x is transposed on the HOST: the kernel receives x^T [C, T] so the
  projection contraction tiles load directly via DMA (no PE transposes).
- The attention core runs in bf16: qt/kt/v/es tiles are bf16 (projection
  matmuls accumulate in fp32 PSUM; DVE bias-add drains cast on write).
  S matmuls use bf16 FWL weight loads; exp output and causal-mask
  multiplies are bf16 ([128,128] diagonal blocks only).
- Diagonal narrowing: for the 4 k-blocks of each q-chunk's diagonal
  512x512 block, S^T / exp / PV only cover queries >= block start
  (shift-packed so each head's valid region stays contiguous).
- Attention is processed per HEAD (not head-pair): one [128,1024] fp32
  PSUM tile per (head, 2-k-block group) gives pipeline depth 2 in 4 banks,
  so the S matmuls of group g+1 never wait on exp of group g.
- Normalization: copy the PV ones-column (softmax denominator) row to
  SBUF, DRAM-bounce it across 64 partitions, DVE fast-reciprocal on the
  [64,512], then one DVE mul deferred to the next head so DMA latency
  never blocks the DVE queue head.
- Out-projection of q-chunk i is deferred two chunks and interleaved as
  PE filler into later attention (as are the next chunk's projections), at
  per-group granularity, so the PE never idles long enough for the HAM
  clock gate to re-throttle.
- Startup: weight tiles stream m-major in 128x128 pieces across all three
  DMA queues so the first projection matmul starts after ~1.25 MB instead
  of 4 MB.
"""

import numpy as np

B = 4
T = 2048
C = 1024
HG = 512          # head channels per core (8 heads x 64)
Dh = 64
NHL = 8           # local heads per core
TCH = 512         # T-chunk (q-chunk) width
NTC = T // TCH    # 4
NCC = C // 128    # 8 contraction chunks for projections
NMT = HG // 128   # 4 row-tiles of q/k channels
VW = NHL * (Dh + 1)   # 520: V tiles with a ones column per head

_CACHE = {}
_DEBUG_DUMP = False


def _build_nc():
    import concourse.bass as bass
    import concourse.bacc as bacc
    import concourse.tile as tile
    import concourse.mybir as mybir

    f32 = mybir.dt.float32
    f32r = mybir.dt.float32r
    bf16 = mybir.dt.bfloat16
    AF = mybir.ActivationFunctionType

    nc = bacc.Bacc("TRN2", target_bir_lowering=False, debug=False,
                   enable_asserts=False)
    # x arrives pre-transposed: [C, T]
    xT_d = nc.dram_tensor("xT", [C, T], f32r, kind="ExternalInput").ap()
    wq_d = nc.dram_tensor("wq", [C, HG], f32r, kind="ExternalInput").ap()
    wk_d = nc.dram_tensor("wk", [C, HG], f32r, kind="ExternalInput").ap()
    wv_d = nc.dram_tensor("wv", [C, HG], f32r, kind="ExternalInput").ap()
    bq_d = nc.dram_tensor("bq", [HG], f32, kind="ExternalInput").ap()
    bk_d = nc.dram_tensor("bk", [HG], f32, kind="ExternalInput").ap()
    bv_d = nc.dram_tensor("bv", [HG], f32, kind="ExternalInput").ap()
    wo_d = nc.dram_tensor("wo", [HG, C], f32r, kind="ExternalInput").ap()
    tr_d = nc.dram_tensor("tri", [128, 128], bf16, kind="ExternalInput").ap()
    vo_d = nc.dram_tensor("vones", [128, NHL], bf16, kind="ExternalInput").ap()
    y_d = nc.dram_tensor("y", [T, C], f32, kind="ExternalOutput").ap()
    # DRAM bounce rows for the per-(chunk, head) denominator broadcast
    rb_d = nc.dram_tensor("rbscratch", [NTC * NHL, TCH], f32,
                          kind="Internal").ap()
    if _DEBUG_DUMP:
        ktd_d = nc.dram_tensor("ktdump", [NMT, 128, T], bf16,
                               kind="ExternalOutput").ap()
        vd_d = nc.dram_tensor("vdump", [T // 128, 128, VW], bf16,
                              kind="ExternalOutput").ap()
        otd_d = nc.dram_tensor("otdump", [NMT, 128, TCH], f32,
                               kind="ExternalOutput").ap()

    def mm(out, lhsT, rhs, start, stop):
        nc.tensor.matmul(out, lhsT, rhs, start=start, stop=stop)

    with tile.TileContext(nc) as tc:
        with tc.tile_pool(name="wp", bufs=1) as wp, \
             tc.tile_pool(name="ktp", bufs=1) as ktp, \
             tc.tile_pool(name="vp", bufs=1) as vp, \
             tc.tile_pool(name="qtp", bufs=2) as qtp, \
             tc.tile_pool(name="xp", bufs=2) as xp, \
             tc.tile_pool(name="esp", bufs=4) as esp, \
             tc.tile_pool(name="rp", bufs=2) as rp, \
             tc.tile_pool(name="otp", bufs=3) as otp, \
             tc.tile_pool(name="yst", bufs=2) as yst, \
             tc.tile_pool(name="psM", bufs=2, space="PSUM") as psM, \
             tc.tile_pool(name="psS", bufs=2, space="PSUM") as psS, \
             tc.tile_pool(name="psO", bufs=2, space="PSUM") as psO:

            # ---- startup loads spread across the three DMA queues so the
            # first Q-projection matmul can start after ~1.25 MB ----
            xin0 = [xp.tile([128, TCH], f32r, name=f"xt{c}", tag=f"xt{c}")
                    for c in range(NCC)]
            for c in range(4):
                nc.sync.dma_start(out=xin0[c],
                                  in_=xT_d[c * 128:(c + 1) * 128, 0:TCH])
            for c in range(4, NCC):
                nc.gpsimd.dma_start(out=xin0[c],
                                    in_=xT_d[c * 128:(c + 1) * 128, 0:TCH])
            wq_sb = [wp.tile([128, HG], f32r, name=f"wq{c}", tag=f"wq{c}")
                     for c in range(NCC)]
            wk_sb = [wp.tile([128, HG], f32r, name=f"wk{c}", tag=f"wk{c}")
                     for c in range(NCC)]
            wv_sb = [wp.tile([128, HG], f32r, name=f"wv{c}", tag=f"wv{c}")
                     for c in range(NCC)]
            wo_sb = [wp.tile([128, C], f32r, name=f"wo{m}", tag=f"wo{m}")
                     for m in range(NMT)]
            # m-major 128x128 pieces so Q-proj m=0 starts after 0.25 MB
            for m in range(NMT):
                for c in range(NCC):
                    nc.scalar.dma_start(
                        out=wq_sb[c][:, m * 128:(m + 1) * 128],
                        in_=wq_d[c * 128:(c + 1) * 128, m * 128:(m + 1) * 128])
            for m in range(NMT):
                for c in range(NCC):
                    nc.sync.dma_start(
                        out=wk_sb[c][:, m * 128:(m + 1) * 128],
                        in_=wk_d[c * 128:(c + 1) * 128, m * 128:(m + 1) * 128])
            for c in range(NCC):
                nc.gpsimd.dma_start(out=wv_sb[c],
                                    in_=wv_d[c * 128:(c + 1) * 128, :])
            for m in range(NMT):
                nc.scalar.dma_start(out=wo_sb[m],
                                    in_=wo_d[m * 128:(m + 1) * 128, :])
            bq_sb = [wp.tile([128, 1], f32, name=f"bq{m}", tag=f"bq{m}")
                     for m in range(NMT)]
            bk_sb = [wp.tile([128, 1], f32, name=f"bk{m}", tag=f"bk{m}")
                     for m in range(NMT)]
            for m in range(NMT):
                nc.gpsimd.dma_start(
                    out=bq_sb[m],
                    in_=bq_d[m * 128:(m + 1) * 128].rearrange("(p o) -> p o", o=1))
                nc.gpsimd.dma_start(
                    out=bk_sb[m],
                    in_=bk_d[m * 128:(m + 1) * 128].rearrange("(p o) -> p o", o=1))
            bv_bc = wp.tile([128, HG], f32, tag="bvbc")
            bv_src = bass.AP(tensor=bv_d.tensor, offset=bv_d.offset,
                             ap=[[0, 128]] + list(bv_d.ap))
            nc.gpsimd.dma_start(out=bv_bc, in_=bv_src)
            tri = wp.tile([128, 128], bf16, tag="tri")
            nc.gpsimd.dma_start(out=tri, in_=tr_d)

            kt_sb = [ktp.tile([128, T], bf16, name=f"kt{m}", tag=f"kt{m}")
                     for m in range(NMT)]
            v_sb = [vp.tile([128, VW], bf16, name=f"v{t}", tag=f"v{t}")
                    for t in range(T // 128)]
            for t in range(T // 128):
                nc.gpsimd.dma_start(
                    out=v_sb[t].rearrange("p (h e) -> p h e", h=NHL)[:, :, Dh:Dh + 1],
                    in_=vo_d.rearrange("p (h e) -> p h e", e=1))

            # ======== fine-grained projection / out-projection units,
            # consumed as PE filler between attention groups ========
            store = {}

            def load_x(ti):
                t0 = ti * TCH
                st = store[ti] = {}
                if ti == 0:
                    st["xt"] = xin0
                else:
                    xt = st["xt"] = [
                        xp.tile([128, TCH], f32r, name=f"xt{c}", tag=f"xt{c}")
                        for c in range(NCC)]
                    for c in range(NCC):
                        nc.sync.dma_start(
                            out=xt[c],
                            in_=xT_d[c * 128:(c + 1) * 128, t0:t0 + TCH])
                st["qt"] = [None] * NMT
                st["ps"] = {}

            def qk_units(ti):
                t0 = ti * TCH
                st = store[ti]

                def u_pq(m, half):
                    def f():
                        if half == 0:
                            pq = st["ps"]["q", m] = psM.tile(
                                [128, TCH], f32, tag="mm", name="pq")
                            for c in range(4):
                                mm(pq, wq_sb[c][:, m * 128:(m + 1) * 128],
                                   st["xt"][c], c == 0, False)
                        else:
                            pq = st["ps"].pop(("q", m))
                            for c in range(4, NCC):
                                mm(pq, wq_sb[c][:, m * 128:(m + 1) * 128],
                                   st["xt"][c], False, c == NCC - 1)
                            qtm = qtp.tile([128, TCH], bf16, name=f"qt{m}",
                                           tag=f"qt{m}")
                            st["qt"][m] = qtm
                            nc.vector.tensor_scalar_add(qtm, pq, bq_sb[m])
                    return f
                for m in range(NMT):
                    yield u_pq(m, 0)
                    yield u_pq(m, 1)

                def u_pk(m, half):
                    def f():
                        if half == 0:
                            pk = st["ps"]["k", m] = psM.tile(
                                [128, TCH], f32, tag="mm", name="pk")
                            for c in range(4):
                                mm(pk, wk_sb[c][:, m * 128:(m + 1) * 128],
                                   st["xt"][c], c == 0, False)
                        else:
                            pk = st["ps"].pop(("k", m))
                            for c in range(4, NCC):
                                mm(pk, wk_sb[c][:, m * 128:(m + 1) * 128],
                                   st["xt"][c], False, c == NCC - 1)
                            nc.vector.tensor_scalar_add(
                                kt_sb[m][:, t0:t0 + TCH], pk, bk_sb[m])
                    return f
                for m in range(NMT):
                    yield u_pk(m, 0)
                    yield u_pk(m, 1)

            def v_units(ti):
                t0 = ti * TCH
                st = store[ti]

                def u_pv(s, half):
                    def f():
                        if half == 0:
                            pv = st["ps"]["v", s] = psM.tile(
                                [128, HG], f32, tag="mm", name="pv")
                            for c in range(4):
                                mm(pv, st["xt"][c][:, s * 128:(s + 1) * 128],
                                   wv_sb[c], c == 0, False)
                        else:
                            pv = st["ps"].pop(("v", s))
                            for c in range(4, NCC):
                                mm(pv, st["xt"][c][:, s * 128:(s + 1) * 128],
                                   wv_sb[c], False, c == NCC - 1)
                            vt = v_sb[(t0 + s * 128) // 128]
                            nc.vector.tensor_add(
                                vt.rearrange("p (h e) -> p h e", h=NHL)[:, :, 0:Dh],
                                pv.rearrange("p (h d) -> p h d", h=NHL),
                                bv_bc.rearrange("p (h d) -> p h d", h=NHL))
                    return f
                for s in range(4):
                    yield u_pv(s, 0)
                    yield u_pv(s, 1)

            def outproj_units(qc):
                ot = store[qc]["ot"]

                def u_py(s, n):
                    def f():
                        py = psM.tile([128, TCH], f32, tag="mm", name="py")
                        for m in range(NMT):
                            mm(py, ot[m][:, s * 128:(s + 1) * 128],
                               wo_sb[m][:, n * TCH:(n + 1) * TCH],
                               m == 0, m == NMT - 1)
                        yt = yst.tile([128, TCH], f32, tag="yst", name="yt")
                        if n == 0:
                            nc.vector.tensor_copy(yt, py)
                        else:
                            nc.scalar.copy(yt, py)
                        nc.sync.dma_start(
                            out=y_d[qc * TCH + s * 128:qc * TCH + (s + 1) * 128,
                                    n * TCH:(n + 1) * TCH],
                            in_=yt)
                    return f
                for s in range(4):
                    for n in range(2):
                        yield u_py(s, n)

            from collections import deque

            def merge_fillers(a, b):
                # spread b's units evenly through a's
                a, b = list(a), list(b)
                if not b:
                    return deque(a)
                out = deque()
                r = max(1, len(a) // (len(b) + 1))
                ai = 0
                for u in b:
                    out.extend(a[ai:ai + r])
                    ai += r
                    out.append(u)
                out.extend(a[ai:])
                return out

            deferred_muls = []

            def emit_deferred():
                while deferred_muls:
                    dst, src, rb = deferred_muls.pop(0)
                    nc.vector.tensor_mul(dst, src, rb)

            # chunk-0 projections up front
            load_x(0)
            for u in qk_units(0):
                u()
            for u in v_units(0):
                u()
            units = deque()

            for ti in range(NTC):
                while units:          # finish chunk ti's projections
                    units.popleft()()
                # filler plan: qc0 <- QKV(1); qc1 <- QKV(2);
                # qc2 <- QKV(3) + outproj(0); qc3 <- outproj(1) + outproj(2)
                if ti + 1 < NTC:
                    load_x(ti + 1)    # start the x DMAs ahead of the fillers
                if ti <= 1:
                    units = deque(list(qk_units(ti + 1)) +
                                  list(v_units(ti + 1)))
                elif ti == 2:
                    units = merge_fillers(
                        list(qk_units(3)) + list(v_units(3)),
                        outproj_units(0))
                else:
                    units = merge_fillers(outproj_units(1), outproj_units(2))

                qc = ti
                nkt = 4 * (qc + 1)
                ngrp = nkt // 2
                qt = store[ti]["qt"]
                ot = store[ti]["ot"] = [
                    otp.tile([128, TCH], f32r, name=f"ot{m}", tag=f"ot{m}")
                    for m in range(NMT)]
                nsteps = NHL * ngrp
                step = 0
                for h in range(NHL):
                    hp, base = h // 2, (h % 2) * 64
                    last_head = (ti == NTC - 1 and h == NHL - 1)
                    emit_deferred()
                    qth = qt[hp][base:base + 64, :]
                    kth = kt_sb[hp][base:base + 64, :]
                    po = psO.tile([128, TCH], f32, tag="o", name="po")
                    pend = None    # software pipeline: PV lags one group
                    for grp in range(ngrp):
                        k0, k1 = 2 * grp, 2 * grp + 1
                        # query offset per k-block (diagonal narrowing)
                        offs = [max(0, 128 * (k - (nkt - 4))) for k in (k0, k1)]
                        ps = psS.tile([128, 2 * TCH], f32, tag="sT", name="ps")
                        for slot, k in enumerate((k0, k1)):
                            off = offs[slot]
                            c0 = slot * TCH
                            mm(ps[:, c0:c0 + TCH - off],
                               kth[:, k * 128:(k + 1) * 128],
                               qth[:, off:TCH], True, True)
                        es = esp.tile([128, 2 * TCH], bf16, tag="es", name="es")
                        if offs[1] == 0:            # fully non-diagonal group
                            rngs = [(0, 2 * TCH)]
                        elif offs[0] == 0:          # dj=0: contiguous [0:896]
                            rngs = [(0, TCH + (TCH - offs[1]))]
                        else:                       # dj=1: two valid ranges
                            rngs = [(0, TCH - offs[0]),
                                    (TCH, TCH + (TCH - offs[1]))]
                        for a, b in rngs:
                            nc.scalar.activation(es[:, a:b], ps[:, a:b],
                                                 AF.Exp, scale=0.125)
                        if k0 >= nkt - 4:           # diagonal: mask leading tri
                            for slot in (0, 1):
                                c0 = slot * TCH
                                nc.vector.tensor_mul(
                                    es[:, c0:c0 + 128], es[:, c0:c0 + 128], tri)
                        if pend is not None:
                            for (kk, oo, cc0, pes) in pend:
                                mm(po[0:Dh + 1, oo:TCH],
                                   v_sb[kk][:, h * 65:(h + 1) * 65],
                                   pes[:, cc0:cc0 + TCH - oo], kk == 0, False)
                        pend = [(k, offs[sl], sl * TCH, es)
                                for sl, k in enumerate((k0, k1))]
                        # PE filler: keep the PE dense while ACT churns exps
                        step += 1
                        nfill = -(-len(units) // max(1, nsteps - step + 1))
                        for _ in range(min(nfill, len(units))):
                            units.popleft()()
                    for (kk, oo, cc0, pes) in pend:
                        mm(po[0:Dh + 1, oo:TCH],
                           v_sb[kk][:, h * 65:(h + 1) * 65],
                           pes[:, cc0:cc0 + TCH - oo], kk == 0, kk == nkt - 1)
                    # normalize: copy the denominator row to SBUF, DRAM-bounce
                    # it across 64 partitions, fast-reciprocal on the full
                    # [64,512] (reciprocal_approx_fast silently corrupts on HW
                    # for single-partition base-64 PSUM reads), then a mul
                    # deferred to the next head so the DMA latency never
                    # blocks the DVE queue head
                    rpt = rp.tile([1, TCH], f32, tag="rp", name="rpt")
                    nc.vector.tensor_copy(rpt, po[Dh:Dh + 1, :])
                    idx = qc * NHL + h
                    row = rb_d[idx:idx + 1, :]
                    nc.sync.dma_start(out=row, in_=rpt)
                    rbr = rp.tile([64, TCH], f32, tag="rbr", name="rbr")
                    bsrc = bass.AP(tensor=row.tensor, offset=row.offset,
                                   ap=[[0, 64]] + list(row.ap)[1:])
                    nc.sync.dma_start(out=rbr, in_=bsrc)
                    rbs = rp.tile([64, TCH], f32, tag="rbs", name="rbs")
                    nc.vector.reciprocal_approx_fast(rbs, rbr)
                    deferred_muls.append(
                        (ot[hp][base:base + 64, :], po[0:Dh, :], rbs))
                    if last_head:
                        emit_deferred()

            while units:
                units.popleft()()
            emit_deferred()
            for u in outproj_units(NTC - 1):
                u()
            if _DEBUG_DUMP:
                for m in range(NMT):
                    nc.sync.dma_start(out=ktd_d[m], in_=kt_sb[m])
                    nc.sync.dma_start(out=otd_d[m],
                                      in_=store[3]["ot"][m].bitcast(f32))
                for t in range(T // 128):
                    nc.sync.dma_start(out=vd_d[t], in_=v_sb[t])
    nc.compile()
    return nc


def _get_nc():
    if "nc" not in _CACHE:
        _CACHE["nc"] = _build_nc()
    return _CACHE["nc"]


def _tri_np():
    import ml_dtypes
    p = np.arange(128)[:, None]
    u = np.arange(128)[None, :]
    return (u >= p).astype(ml_dtypes.bfloat16)


def _in_maps(x, w_qkv, b_qkv, w_out):
    import ml_dtypes
    tri = _tri_np()
    vones = np.ones((128, NHL), dtype=ml_dtypes.bfloat16)
    maps = []
    for c in range(8):
        b, g = c // 2, c % 2
        s = g * HG
        maps.append({
            "xT": np.ascontiguousarray(x[b].T),
            "wq": np.ascontiguousarray(w_qkv[:, s:s + HG]),
            "wk": np.ascontiguousarray(w_qkv[:, C + s:C + s + HG]),
            "wv": np.ascontiguousarray(w_qkv[:, 2 * C + s:2 * C + s + HG]),
            "bq": np.ascontiguousarray(b_qkv[s:s + HG]),
            "bk": np.ascontiguousarray(b_qkv[C + s:C + s + HG]),
            "bv": np.ascontiguousarray(b_qkv[2 * C + s:2 * C + s + HG]),
            "wo": np.ascontiguousarray(w_out[s:s + HG, :]),
            "tri": tri,
            "vones": vones,
        })
    return maps


def _run(x, w_qkv, b_qkv, w_out, b_out, trace=False, tmpdir=None):
    from concourse import bass_utils
    nc = _get_nc()
    maps = _in_maps(x, w_qkv, b_qkv, w_out)
    # the device occasionally reports a transient unrecoverable-exec error
    # right after a reset; one retry clears it
    last = None
    for attempt in range(3):
        try:
            res = bass_utils.run_bass_kernel_spmd(
                nc, maps, core_ids=list(range(8)), trace=trace, tmpdir=tmpdir)
            break
        except Exception as e:
            last = e
            if attempt == 2:
                raise
    else:
        raise last
    ys = [res.results[c]["y"] for c in range(8)]
    out = np.stack([ys[2 * b] + ys[2 * b + 1] for b in range(B)])
    out += np.asarray(b_out, dtype=np.float32)[None, None, :]
    return out.astype(np.float32), res


def kernel(x, w_qkv, b_qkv, w_out, b_out):
    x = np.asarray(x, dtype=np.float32)
    w_qkv = np.asarray(w_qkv, dtype=np.float32)
    b_qkv = np.asarray(b_qkv, dtype=np.float32)
    w_out = np.asarray(w_out, dtype=np.float32)
    b_out = np.asarray(b_out, dtype=np.float32)
    out, _ = _run(x, w_qkv, b_qkv, w_out, b_out, trace=False)
    return out


# revision 19
# speedup vs baseline: 1.4530x; 1.0757x over previous
"""Causal self-attention on 8 trn2 NeuronCores.

Sharding: core c = (batch b = c//2, head-group g = c%2). Each core computes
QKV projection for its 8 heads of its batch, causal flash-attention in a
transposed (S^T) layout, and a partial out-projection (its 512 rows of
w_out). Host sums the two partials per batch and adds b_out.

v4 design:
- x is transposed on the HOST: the kernel receives x^T [C, T] so the
  projection contraction tiles load directly via DMA (no PE transposes).
- The attention core runs in bf16: qt/kt/v/es tiles are bf16 (projection
  matmuls accumulate in fp32 PSUM; DVE bias-add drains cast on write).
  S matmuls use bf16 FWL weight loads; exp output and causal-mask
  multiplies are bf16 ([128,128] diagonal blocks only).
- Diagonal narrowing: for the 4 k-blocks of each q-chunk's diagonal
  512x512 block, S^T / exp / PV only cover queries >= block start
  (shift-packed so each head's valid region stays contiguous).
- Attention is processed per HEAD (not head-pair): one [128,1024] fp32
  PSUM tile per (head, 2-k-block group) gives pipeline depth 2 in 4 banks,
  so the S matmuls of group g+1 never wait on exp of group g.
- Normalization: copy the PV ones-column (softmax denominator) row to
  SBUF, DRAM-bounce it across 64 partitions, DVE fast-reciprocal on the
  [64,512], then one DVE mul deferred to the next head so DMA latency
  never blocks the DVE queue head.
- Out-projection of q-chunk i is deferred two chunks and interleaved as
  PE filler into later attention (as are the next chunk's projections), at
  per-group granularity, so the PE never idles long enough for the HAM
  clock gate to re-throttle.
- Startup: weight tiles stream m-major in 128x128 pieces across all three
  DMA queues so the first projection matmul starts after ~1.25 MB instead
  of 4 MB.
"""

import numpy as np

B = 4
T = 2048
C = 1024
HG = 512          # head channels per core (8 heads x 64)
Dh = 64
NHL = 8           # local heads per core
TCH = 512         # T-chunk (q-chunk) width
NTC = T // TCH    # 4
NCC = C // 128    # 8 contraction chunks for projections
NMT = HG // 128   # 4 row-tiles of q/k channels
VW = NHL * (Dh + 1)   # 520: V tiles with a ones column per head

_CACHE = {}
_DEBUG_DUMP = False


def _build_nc():
    import concourse.bass as bass
    import concourse.bacc as bacc
    import concourse.tile as tile
    import concourse.mybir as mybir

    f32 = mybir.dt.float32
    f32r = mybir.dt.float32r
    bf16 = mybir.dt.bfloat16
    AF = mybir.ActivationFunctionType

    nc = bacc.Bacc("TRN2", target_bir_lowering=False, debug=False,
                   enable_asserts=False)
    # x arrives pre-transposed and pre-cast: [C, T] bf16
    xT_d = nc.dram_tensor("xT", [C, T], bf16, kind="ExternalInput").ap()
    wq_d = nc.dram_tensor("wq", [C, HG], bf16, kind="ExternalInput").ap()
    wk_d = nc.dram_tensor("wk", [C, HG], bf16, kind="ExternalInput").ap()
    wv_d = nc.dram_tensor("wv", [C, HG], bf16, kind="ExternalInput").ap()
    bq_d = nc.dram_tensor("bq", [HG], f32, kind="ExternalInput").ap()
    bk_d = nc.dram_tensor("bk", [HG], f32, kind="ExternalInput").ap()
    bv_d = nc.dram_tensor("bv", [HG], f32, kind="ExternalInput").ap()
    wo_d = nc.dram_tensor("wo", [HG, C], f32r, kind="ExternalInput").ap()
    tr_d = nc.dram_tensor("tri", [128, 128], bf16, kind="ExternalInput").ap()
    on_d = nc.dram_tensor("ones", [1, 64], f32r, kind="ExternalInput").ap()
    vo_d = nc.dram_tensor("vones", [128, NHL], bf16, kind="ExternalInput").ap()
    y_d = nc.dram_tensor("y", [T, C], f32, kind="ExternalOutput").ap()
    # DRAM bounce rows for the per-(chunk, head) denominator broadcast
    rb_d = nc.dram_tensor("rbscratch", [NTC * NHL, TCH], f32,
                          kind="Internal").ap()
    if _DEBUG_DUMP:
        ktd_d = nc.dram_tensor("ktdump", [NMT, 128, T], bf16,
                               kind="ExternalOutput").ap()
        vd_d = nc.dram_tensor("vdump", [T // 128, 128, VW], bf16,
                              kind="ExternalOutput").ap()
        otd_d = nc.dram_tensor("otdump", [NMT, 128, TCH], f32,
                               kind="ExternalOutput").ap()

    def mm(out, lhsT, rhs, start, stop):
        nc.tensor.matmul(out, lhsT, rhs, start=start, stop=stop)

    with tile.TileContext(nc) as tc:
        with tc.tile_pool(name="wp", bufs=1) as wp, \
             tc.tile_pool(name="ktp", bufs=1) as ktp, \
             tc.tile_pool(name="vp", bufs=1) as vp, \
             tc.tile_pool(name="qtp", bufs=2) as qtp, \
             tc.tile_pool(name="xp", bufs=2) as xp, \
             tc.tile_pool(name="esp", bufs=4) as esp, \
             tc.tile_pool(name="rp", bufs=2) as rp, \
             tc.tile_pool(name="otp", bufs=3) as otp, \
             tc.tile_pool(name="yst", bufs=2) as yst, \
             tc.tile_pool(name="psM", bufs=2, space="PSUM") as psM, \
             tc.tile_pool(name="psS", bufs=2, space="PSUM") as psS, \
             tc.tile_pool(name="psO", bufs=2, space="PSUM") as psO:

            # ---- startup loads spread across the three DMA queues so the
            # first Q-projection matmul can start after ~1.25 MB ----
            xin0 = [xp.tile([128, TCH], bf16, name=f"xt{c}", tag=f"xt{c}")
                    for c in range(NCC)]
            for c in range(4):
                nc.sync.dma_start(out=xin0[c],
                                  in_=xT_d[c * 128:(c + 1) * 128, 0:TCH])
            for c in range(4, NCC):
                nc.gpsimd.dma_start(out=xin0[c],
                                    in_=xT_d[c * 128:(c + 1) * 128, 0:TCH])
            wq_sb = [wp.tile([128, HG], bf16, name=f"wq{c}", tag=f"wq{c}")
                     for c in range(NCC)]
            wk_sb = [wp.tile([128, HG], bf16, name=f"wk{c}", tag=f"wk{c}")
                     for c in range(NCC)]
            wv_sb = [wp.tile([128, HG], bf16, name=f"wv{c}", tag=f"wv{c}")
                     for c in range(NCC)]
            wo_sb = [wp.tile([128, C], f32r, name=f"wo{m}", tag=f"wo{m}")
                     for m in range(NMT)]
            # m-major 128x128 pieces so Q-proj m=0 starts after 0.25 MB
            for m in range(NMT):
                for c in range(NCC):
                    nc.scalar.dma_start(
                        out=wq_sb[c][:, m * 128:(m + 1) * 128],
                        in_=wq_d[c * 128:(c + 1) * 128, m * 128:(m + 1) * 128])
            for m in range(NMT):
                for c in range(NCC):
                    nc.sync.dma_start(
                        out=wk_sb[c][:, m * 128:(m + 1) * 128],
                        in_=wk_d[c * 128:(c + 1) * 128, m * 128:(m + 1) * 128])
            for c in range(NCC):
                nc.gpsimd.dma_start(out=wv_sb[c],
                                    in_=wv_d[c * 128:(c + 1) * 128, :])
            for m in range(NMT):
                nc.scalar.dma_start(out=wo_sb[m],
                                    in_=wo_d[m * 128:(m + 1) * 128, :])
            bq_sb = [wp.tile([128, 1], f32, name=f"bq{m}", tag=f"bq{m}")
                     for m in range(NMT)]
            bk_sb = [wp.tile([128, 1], f32, name=f"bk{m}", tag=f"bk{m}")
                     for m in range(NMT)]
            for m in range(NMT):
                nc.gpsimd.dma_start(
                    out=bq_sb[m],
                    in_=bq_d[m * 128:(m + 1) * 128].rearrange("(p o) -> p o", o=1))
                nc.gpsimd.dma_start(
                    out=bk_sb[m],
                    in_=bk_d[m * 128:(m + 1) * 128].rearrange("(p o) -> p o", o=1))
            bv_bc = wp.tile([128, HG], f32, tag="bvbc")
            bv_src = bass.AP(tensor=bv_d.tensor, offset=bv_d.offset,
                             ap=[[0, 128]] + list(bv_d.ap))
            nc.gpsimd.dma_start(out=bv_bc, in_=bv_src)
            tri = wp.tile([128, 128], bf16, tag="tri")
            nc.gpsimd.dma_start(out=tri, in_=tr_d)
            ones = wp.tile([1, 64], f32r, tag="ones")
            nc.gpsimd.dma_start(out=ones, in_=on_d)

            kt_sb = [ktp.tile([128, T], bf16, name=f"kt{m}", tag=f"kt{m}")
                     for m in range(NMT)]
            v_sb = [vp.tile([128, VW], bf16, name=f"v{t}", tag=f"v{t}")
                    for t in range(T // 128)]
            for t in range(T // 128):
                nc.gpsimd.dma_start(
                    out=v_sb[t].rearrange("p (h e) -> p h e", h=NHL)[:, :, Dh:Dh + 1],
                    in_=vo_d.rearrange("p (h e) -> p h e", e=1))

            # ======== fine-grained projection / out-projection units,
            # consumed as PE filler between attention groups ========
            store = {}

            def load_x(ti):
                t0 = ti * TCH
                st = store[ti] = {}
                if ti == 0:
                    st["xt"] = xin0
                else:
                    xt = st["xt"] = [
                        xp.tile([128, TCH], bf16, name=f"xt{c}", tag=f"xt{c}")
                        for c in range(NCC)]
                    for c in range(NCC):
                        nc.sync.dma_start(
                            out=xt[c],
                            in_=xT_d[c * 128:(c + 1) * 128, t0:t0 + TCH])
                st["qt"] = [None] * NMT
                st["ps"] = {}

            def qk_units(ti):
                t0 = ti * TCH
                st = store[ti]

                def u_pq(m, half):
                    def f():
                        if half == 0:
                            pq = st["ps"]["q", m] = psM.tile(
                                [128, TCH], f32, tag="mm", name="pq")
                            for c in range(4):
                                mm(pq, wq_sb[c][:, m * 128:(m + 1) * 128],
                                   st["xt"][c], c == 0, False)
                        else:
                            pq = st["ps"].pop(("q", m))
                            for c in range(4, NCC):
                                mm(pq, wq_sb[c][:, m * 128:(m + 1) * 128],
                                   st["xt"][c], False, c == NCC - 1)
                            qtm = qtp.tile([128, TCH], bf16, name=f"qt{m}",
                                           tag=f"qt{m}")
                            st["qt"][m] = qtm
                            nc.vector.tensor_scalar_add(qtm, pq, bq_sb[m])
                    return f
                for m in range(NMT):
                    yield u_pq(m, 0)
                    yield u_pq(m, 1)

                def u_pk(m, half):
                    def f():
                        if half == 0:
                            pk = st["ps"]["k", m] = psM.tile(
                                [128, TCH], f32, tag="mm", name="pk")
                            for c in range(4):
                                mm(pk, wk_sb[c][:, m * 128:(m + 1) * 128],
                                   st["xt"][c], c == 0, False)
                        else:
                            pk = st["ps"].pop(("k", m))
                            for c in range(4, NCC):
                                mm(pk, wk_sb[c][:, m * 128:(m + 1) * 128],
                                   st["xt"][c], False, c == NCC - 1)
                            nc.vector.tensor_scalar_add(
                                kt_sb[m][:, t0:t0 + TCH], pk, bk_sb[m])
                    return f
                for m in range(NMT):
                    yield u_pk(m, 0)
                    yield u_pk(m, 1)

            def v_units(ti):
                t0 = ti * TCH
                st = store[ti]

                def u_pv(s, half):
                    def f():
                        if half == 0:
                            pv = st["ps"]["v", s] = psM.tile(
                                [128, HG], f32, tag="mm", name="pv")
                            for c in range(4):
                                mm(pv, st["xt"][c][:, s * 128:(s + 1) * 128],
                                   wv_sb[c], c == 0, False)
                        else:
                            pv = st["ps"].pop(("v", s))
                            for c in range(4, NCC):
                                mm(pv, st["xt"][c][:, s * 128:(s + 1) * 128],
                                   wv_sb[c], False, c == NCC - 1)
                            vt = v_sb[(t0 + s * 128) // 128]
                            nc.vector.tensor_add(
                                vt.rearrange("p (h e) -> p h e", h=NHL)[:, :, 0:Dh],
                                pv.rearrange("p (h d) -> p h d", h=NHL),
                                bv_bc.rearrange("p (h d) -> p h d", h=NHL))
                    return f
                for s in range(4):
                    yield u_pv(s, 0)
                    yield u_pv(s, 1)

            def outproj_units(qc):
                ot = store[qc]["ot"]

                def u_py(s, n):
                    def f():
                        py = psM.tile([128, TCH], f32, tag="mm", name="py")
                        for m in range(NMT):
                            mm(py, ot[m][:, s * 128:(s + 1) * 128],
                               wo_sb[m][:, n * TCH:(n + 1) * TCH],
                               m == 0, m == NMT - 1)
                        yt = yst.tile([128, TCH], f32, tag="yst", name="yt")
                        if n == 0:
                            nc.vector.tensor_copy(yt, py)
                        else:
                            nc.scalar.copy(yt, py)
                        nc.sync.dma_start(
                            out=y_d[qc * TCH + s * 128:qc * TCH + (s + 1) * 128,
                                    n * TCH:(n + 1) * TCH],
                            in_=yt)
                    return f
                for s in range(4):
                    for n in range(2):
                        yield u_py(s, n)

            from collections import deque

            def merge_fillers(a, b):
                # spread b's units evenly through a's
                a, b = list(a), list(b)
                if not b:
                    return deque(a)
                out = deque()
                r = max(1, len(a) // (len(b) + 1))
                ai = 0
                for u in b:
                    out.extend(a[ai:ai + r])
                    ai += r
                    out.append(u)
                out.extend(a[ai:])
                return out

            deferred_muls = []

            def emit_deferred():
                while deferred_muls:
                    dst, src, rb = deferred_muls.pop(0)
                    nc.vector.tensor_mul(dst, src, rb)

            # chunk-0 projections up front
            load_x(0)
            for u in qk_units(0):
                u()
            for u in v_units(0):
                u()
            units = deque()

            for ti in range(NTC):
                while units:          # finish chunk ti's projections
                    units.popleft()()
                # filler plan: qc0 <- QKV(1); qc1 <- QKV(2);
                # qc2 <- QKV(3) + outproj(0); qc3 <- outproj(1) + outproj(2)
                if ti + 1 < NTC:
                    load_x(ti + 1)    # start the x DMAs ahead of the fillers
                if ti <= 1:
                    units = deque(list(qk_units(ti + 1)) +
                                  list(v_units(ti + 1)))
                elif ti == 2:
                    units = merge_fillers(
                        list(qk_units(3)) + list(v_units(3)),
                        outproj_units(0))
                else:
                    units = merge_fillers(outproj_units(1), outproj_units(2))

                qc = ti
                nkt = 4 * (qc + 1)
                ngrp = nkt // 2
                qt = store[ti]["qt"]
                ot = store[ti]["ot"] = [
                    otp.tile([128, TCH], f32r, name=f"ot{m}", tag=f"ot{m}")
                    for m in range(NMT)]
                nsteps = NHL * ngrp
                step = 0
                for h in range(NHL):
                    hp, base = h // 2, (h % 2) * 64
                    last_head = (ti == NTC - 1 and h == NHL - 1)
                    emit_deferred()
                    qth = qt[hp][base:base + 64, :]
                    kth = kt_sb[hp][base:base + 64, :]
                    po = psO.tile([128, TCH], f32, tag="o", name="po")
                    pend = None    # software pipeline: PV lags one group
                    for grp in range(ngrp):
                        k0, k1 = 2 * grp, 2 * grp + 1
                        # query offset per k-block (diagonal narrowing)
                        offs = [max(0, 128 * (k - (nkt - 4))) for k in (k0, k1)]
                        ps = psS.tile([128, 2 * TCH], f32, tag="sT", name="ps")
                        for slot, k in enumerate((k0, k1)):
                            off = offs[slot]
                            c0 = slot * TCH
                            mm(ps[:, c0:c0 + TCH - off],
                               kth[:, k * 128:(k + 1) * 128],
                               qth[:, off:TCH], True, True)
                        es = esp.tile([128, 2 * TCH], bf16, tag="es", name="es")
                        if offs[1] == 0:            # fully non-diagonal group
                            rngs = [(0, 2 * TCH)]
                        elif offs[0] == 0:          # dj=0: contiguous [0:896]
                            rngs = [(0, TCH + (TCH - offs[1]))]
                        else:                       # dj=1: two valid ranges
                            rngs = [(0, TCH - offs[0]),
                                    (TCH, TCH + (TCH - offs[1]))]
                        for a, b in rngs:
                            nc.scalar.activation(es[:, a:b], ps[:, a:b],
                                                 AF.Exp, scale=0.125)
                        if k0 >= nkt - 4:           # diagonal: mask leading tri
                            for slot in (0, 1):
                                c0 = slot * TCH
                                nc.vector.tensor_mul(
                                    es[:, c0:c0 + 128], es[:, c0:c0 + 128], tri)
                        if pend is not None:
                            for (kk, oo, cc0, pes) in pend:
                                mm(po[0:Dh + 1, oo:TCH],
                                   v_sb[kk][:, h * 65:(h + 1) * 65],
                                   pes[:, cc0:cc0 + TCH - oo], kk == 0, False)
                        pend = [(k, offs[sl], sl * TCH, es)
                                for sl, k in enumerate((k0, k1))]
                        # PE filler: keep the PE dense while ACT churns exps
                        step += 1
                        nfill = -(-len(units) // max(1, nsteps - step + 1))
                        for _ in range(min(nfill, len(units))):
                            units.popleft()()
                    for (kk, oo, cc0, pes) in pend:
                        mm(po[0:Dh + 1, oo:TCH],
                           v_sb[kk][:, h * 65:(h + 1) * 65],
                           pes[:, cc0:cc0 + TCH - oo], kk == 0, kk == nkt - 1)
                    # normalize: copy the denominator row to SBUF, DRAM-bounce
                    # it across 64 partitions, fast-reciprocal on the full
                    # [64,512] (reciprocal_approx_fast silently corrupts on HW
                    # for single-partition base-64 PSUM reads), then a mul
                    # deferred to the next head so the DMA latency never
                    # blocks the DVE queue head
                    rpt = rp.tile([1, TCH], f32r, tag="rp", name="rpt")
                    nc.vector.tensor_copy(rpt, po[Dh:Dh + 1, :])
                    rbs = rp.tile([64, TCH], f32, tag="rbs", name="rbs")
                    if ti == NTC - 1 and h >= NHL - 2:
                        # tail: broadcast via PE (no DMA round-trip latency)
                        rb = psM.tile([128, TCH], f32, tag="mm", name="rb")
                        mm(rb[0:64, :], ones, rpt, True, True)
                        nc.vector.reciprocal_approx_fast(rbs, rb[0:64, :])
                    else:
                        idx = qc * NHL + h
                        row = rb_d[idx:idx + 1, :]
                        nc.sync.dma_start(out=row, in_=rpt.bitcast(f32))
                        rbr = rp.tile([64, TCH], f32, tag="rbr", name="rbr")
                        bsrc = bass.AP(tensor=row.tensor, offset=row.offset,
                                       ap=[[0, 64]] + list(row.ap)[1:])
                        nc.sync.dma_start(out=rbr, in_=bsrc)
                        nc.vector.reciprocal_approx_fast(rbs, rbr)
                    deferred_muls.append(
                        (ot[hp][base:base + 64, :], po[0:Dh, :], rbs))
                    if last_head:
                        emit_deferred()

            while units:
                units.popleft()()
            emit_deferred()
            for u in outproj_units(NTC - 1):
                u()
            if _DEBUG_DUMP:
                for m in range(NMT):
                    nc.sync.dma_start(out=ktd_d[m], in_=kt_sb[m])
                    nc.sync.dma_start(out=otd_d[m],
                                      in_=store[3]["ot"][m].bitcast(f32))
                for t in range(T // 128):
                    nc.sync.dma_start(out=vd_d[t], in_=v_sb[t])
    nc.compile()
    return nc


def _get_nc():
    if "nc" not in _CACHE:
        _CACHE["nc"] = _build_nc()
    return _CACHE["nc"]


def _tri_np():
    import ml_dtypes
    p = np.arange(128)[:, None]
    u = np.arange(128)[None, :]
    return (u >= p).astype(ml_dtypes.bfloat16)


def _in_maps(x, w_qkv, b_qkv, w_out):
    import ml_dtypes
    bf = ml_dtypes.bfloat16
    tri = _tri_np()
    vones = np.ones((128, NHL), dtype=bf)
    maps = []
    for c in range(8):
        b, g = c // 2, c % 2
        s = g * HG
        maps.append({
            "xT": np.ascontiguousarray(x[b].T).astype(bf),
            "wq": np.ascontiguousarray(w_qkv[:, s:s + HG]).astype(bf),
            "wk": np.ascontiguousarray(w_qkv[:, C + s:C + s + HG]).astype(bf),
            "wv": np.ascontiguousarray(
                w_qkv[:, 2 * C + s:2 * C + s + HG]).astype(bf),
            "bq": np.ascontiguousarray(b_qkv[s:s + HG]),
            "bk": np.ascontiguousarray(b_qkv[C + s:C + s + HG]),
            "bv": np.ascontiguousarray(b_qkv[2 * C + s:2 * C + s + HG]),
            "wo": np.ascontiguousarray(w_out[s:s + HG, :]),
            "tri": tri,
            "vones": vones,
            "ones": np.ones((1, 64), dtype=np.float32),
        })
    return maps


def _run(x, w_qkv, b_qkv, w_out, b_out, trace=False, tmpdir=None):
    from concourse import bass_utils
    nc = _get_nc()
    maps = _in_maps(x, w_qkv, b_qkv, w_out)
    # the device occasionally reports a transient unrecoverable-exec error
    # right after a reset; one retry clears it
    last = None
    for attempt in range(3):
        try:
            res = bass_utils.run_bass_kernel_spmd(
                nc, maps, core_ids=list(range(8)), trace=trace, tmpdir=tmpdir)
            break
        except Exception as e:
            last = e
            if attempt == 2:
                raise
    else:
        raise last
    ys = [res.results[c]["y"] for c in range(8)]
    out = np.stack([ys[2 * b] + ys[2 * b + 1] for b in range(B)])
    out += np.asarray(b_out, dtype=np.float32)[None, None, :]
    return out.astype(np.float32), res


def kernel(x, w_qkv, b_qkv, w_out, b_out):
    x = np.asarray(x, dtype=np.float32)
    w_qkv = np.asarray(w_qkv, dtype=np.float32)
    b_qkv = np.asarray(b_qkv, dtype=np.float32)
    w_out = np.asarray(w_out, dtype=np.float32)
    b_out = np.asarray(b_out, dtype=np.float32)
    out, _ = _run(x, w_qkv, b_qkv, w_out, b_out, trace=False)
    return out
